# revision 1
# baseline (speedup 1.0000x reference)
"""v7: ALL matmuls row-tiled as K=64 pairs into SEPARATE PSUM banks
(concurrent PE array halves, the hardware-correct pattern); AV A/B partial
sums merged on DVE in two single-PSUM-input ops (copy then add) to satisfy
the walrus one-PSUM-operand rule. Projections/out-proj stay K=128 serial
chains. Softmax z rides the AV ones-column; bf16 K=64 broadcast matmul.

PSUM (8 banks): sc 2x[128,2,512]=4 + work 2x[128,2,512]=4 (AV chains, proj
chains plane-0, z-broadcast, out-proj share the work tag rotation).
"""

import numpy as np
from contextlib import ExitStack

EMB = 1024
NH_LOCAL = 8
NPAIR = 4
S_FULL = 2048
NCORES = 8
SCALE = float(np.sqrt(np.float32(EMB)))  # 32.0


def build_nc(S=S_FULL, reps=1):
    import concourse.bacc as bacc
    import concourse.tile as tile
    from concourse import mybir

    bf = mybir.dt.bfloat16
    f32 = mybir.dt.float32
    Act = mybir.ActivationFunctionType
    add = mybir.AluOpType.add

    F = 512
    EC = EMB // 128
    T16 = S // 128
    TB = S // 512
    KC = S // 128

    nc = bacc.Bacc("TRN2", target_bir_lowering=False, debug=False)

    aT_d = nc.dram_tensor("aT", [EC, 128, S], bf, kind="ExternalInput")
    wq_d = nc.dram_tensor("wq", [EC, 128, F], bf, kind="ExternalInput")
    wk_d = nc.dram_tensor("wk", [EC, 128, F], bf, kind="ExternalInput")
    wv_d = nc.dram_tensor("wv", [EC, 128, F], bf, kind="ExternalInput")
    wo_d = nc.dram_tensor("wo", [NPAIR, 128, EMB], bf, kind="ExternalInput")
    bq_d = nc.dram_tensor("bq", [128, NPAIR], f32, kind="ExternalInput")
    out_d = nc.dram_tensor("out", [S, EMB], f32, kind="ExternalOutput")

    with ExitStack() as top:
        tc = top.enter_context(tile.TileContext(nc))
        sb = top.enter_context(tc.tile_pool(name="sb", bufs=1))
        ps = top.enter_context(tc.tile_pool(name="ps", bufs=1, space="PSUM"))

        wo_sb = sb.tile([128, NPAIR, EMB], bf)
        bq_sb = sb.tile([128, NPAIR], f32)
        ones_b = sb.tile([128, 65], bf)
        zr2 = sb.tile([128, 2, 512], bf)
        qt_sb = sb.tile([128, NPAIR, S], bf)
        kt_sb = sb.tile([128, NPAIR, S], bf)
        v_sb = sb.tile([128, T16, NH_LOCAL, 65], bf)
        ctxT = sb.tile([128, NPAIR, S], bf)
        aT = sb.tile([128, EC, S], bf)
        wq_sb = sb.tile([128, EC, F], bf)
        wk_sb = sb.tile([128, EC, F], bf)
        wv_sb = sb.tile([128, EC, F], bf)

        nc.vector.memset(ones_b[:], 1.0)
        nc.vector.memset(zr2[:], 0.0)
        nc.vector.memset(v_sb[:, :, :, 64:65], 1.0)
        for c in range(EC):
            nc.sync.dma_start(aT[:, c, :], aT_d[c])
            nc.sync.dma_start(wk_sb[:, c, :], wk_d[c])
        for c in range(EC):
            nc.sync.dma_start(wq_sb[:, c, :], wq_d[c])
            nc.sync.dma_start(wv_sb[:, c, :], wv_d[c])
        for p in range(NPAIR):
            nc.sync.dma_start(wo_sb[:, p, :], wo_d[p])
        nc.sync.dma_start(bq_sb[:], bq_d[:])

        A = slice(0, 64)
        B = slice(64, 128)

        def chain128(wk_tile, lhs_fn, rhs_fn, n):
            """K=64 A/B half-chains into planes 0/1 (concurrent PE halves)."""
            for c in range(n):
                for j, hs in ((0, A), (1, B)):
                    nc.tensor.matmul(
                        wk_tile[:, j, :],
                        lhs_fn(c)[hs],
                        rhs_fn(c)[hs],
                        start=(c == 0),
                        stop=(c == n - 1),
                    )

        def emit_vproj(t):
            wk = ps.tile([128, 2, 512], f32, tag="work", bufs=2, name="wk")
            chain128(
                wk,
                lambda c: aT[:, c, t * 128 : (t + 1) * 128],
                lambda c: wv_sb[:, c, :],
                EC,
            )
            nc.vector.tensor_copy(
                v_sb[:, t, :, 0:64],
                wk[:, 0, :].rearrange("p (h d) -> p h d", h=NH_LOCAL),
            )
            nc.vector.tensor_add(
                v_sb[:, t, :, 0:64],
                v_sb[:, t, :, 0:64],
                wk[:, 1, :].rearrange("p (h d) -> p h d", h=NH_LOCAL),
            )

        def emit_qproj(tb, p):
            ts = slice(tb * 512, (tb + 1) * 512)
            wk = ps.tile([128, 2, 512], f32, tag="work", bufs=2, name="wk")
            chain128(
                wk,
                lambda c: wq_sb[:, c, p * 128 : (p + 1) * 128],
                lambda c: aT[:, c, ts],
                EC,
            )
            nc.vector.tensor_scalar_add(
                qt_sb[:, p, ts], wk[:, 0, :], bq_sb[:, p : p + 1]
            )
            nc.vector.tensor_add(qt_sb[:, p, ts], qt_sb[:, p, ts], wk[:, 1, :])

        def emit_kproj(tb, p):
            ts = slice(tb * 512, (tb + 1) * 512)
            wk = ps.tile([128, 2, 512], f32, tag="work", bufs=2, name="wk")
            chain128(
                wk,
                lambda c: wk_sb[:, c, p * 128 : (p + 1) * 128],
                lambda c: aT[:, c, ts],
                EC,
            )
            nc.vector.tensor_copy(kt_sb[:, p, ts], wk[:, 0, :])
            nc.vector.tensor_add(kt_sb[:, p, ts], kt_sb[:, p, ts], wk[:, 1, :])

        def emit_score_group(tb, p, kc, exps):
            ts = slice(tb * 512, (tb + 1) * 512)
            ks = slice(kc * 128, (kc + 1) * 128)
            sc = ps.tile([128, 2, 512], f32, tag="sc", bufs=2)
            nc.tensor.matmul(
                sc[:, 0, :], kt_sb[A, p, ks], qt_sb[A, p, ts], start=True, stop=True
            )
            nc.tensor.matmul(
                sc[:, 1, :], kt_sb[B, p, ks], qt_sb[B, p, ts], start=True, stop=True
            )
            nc.scalar.activation(
                exps[:, kc, :, :], sc[:], Act.Exp, scale=1.0 / SCALE
            )

        def emit_av_chunk(p, hh, exps, av, k0, k1):
            h = 2 * p + hh
            for kc in range(k0, k1):
                for j, hs in ((0, A), (1, B)):
                    nc.tensor.matmul(
                        av[0:65, j, :],
                        v_sb[hs, kc, h, :],
                        exps[hs, kc, hh, :],
                        start=(kc == 0),
                        stop=(kc == KC - 1),
                    )

        def emit_av_merge(av, hh):
            """avs = av.planeA + av.planeB via two 1-PSUM-input DVE ops."""
            avs = sb.tile([65, 512], f32, tag="avs", bufs=2)
            nc.vector.tensor_copy(avs[:], av[0:65, 0, :])
            nc.vector.tensor_add(avs[:], avs[:], av[0:65, 1, :])
            nc.vector.tensor_copy(zr2[64:65, hh, :], avs[64:65, :])
            return avs

        def emit_norm(tb, p, avs0, avs1):
            ts = slice(tb * 512, (tb + 1) * 512)
            zw = ps.tile([128, 2, 512], f32, tag="work", bufs=2, name="zw")
            for hh in range(2):
                nc.tensor.matmul(
                    zw[0:65, hh, :],
                    ones_b[B, 0:65],
                    zr2[B, hh, :],
                    start=True,
                    stop=True,
                )
            rb = sb.tile([64, 2, 512], f32, tag="rb", bufs=1)
            for hh in range(2):
                nc.vector.reciprocal(rb[:, hh, :], zw[0:64, hh, :])
            for hh, avs in ((0, avs0), (1, avs1)):
                nc.vector.tensor_mul(
                    ctxT[hh * 64 : (hh + 1) * 64, p, ts],
                    avs[0:64, :],
                    rb[:, hh, :],
                )

        def emit_outproj(tb, ti_local, eb):
            t = tb * 4 + ti_local
            wk = ps.tile([128, 2, 512], f32, tag="work", bufs=2, name="wk")
            chain128(
                wk,
                lambda p_: ctxT[:, p_, t * 128 : (t + 1) * 128],
                lambda p_: wo_sb[:, p_, eb * 512 : (eb + 1) * 512],
                NPAIR,
            )
            o_sb = sb.tile([128, 512], f32, tag="ost", bufs=2)
            nc.vector.tensor_copy(o_sb[:], wk[:, 0, :])
            nc.vector.tensor_add(o_sb[:], o_sb[:], wk[:, 1, :])
            nc.sync.dma_start(
                out_d[t * 128 : (t + 1) * 128, eb * 512 : (eb + 1) * 512],
                o_sb[:],
            )

        for _rep in range(reps):
            for p in range(NPAIR):
                emit_kproj(0, p)
            for p in range(NPAIR):
                emit_qproj(0, p)

            steps = [(tb, p) for tb in range(TB) for p in range(NPAIR)]
            po_sched = {1: [(0, 0), (0, 1), (1, 0)], 2: [(1, 1), (2, 0), (2, 1)],
                        3: [(3, 0), (3, 1)]}
            av1_rng = {8: (0, 3), 9: (3, 6), 10: (6, 9), 11: (9, 12),
                       12: (12, 14), 13: (14, 16)}
            prev = None
            for s, (tb, p) in enumerate(steps):
                exps_cur = sb.tile([128, KC, 2, 512], bf, tag="exps", bufs=2)
                av0 = av1 = avs0 = avs1 = None
                for g in range(16):
                    if s == 0 and g in (4, 8, 12):
                        for pp_ in range(NPAIR):
                            emit_kproj(g // 4, pp_)
                    emit_score_group(tb, p, g, exps_cur)
                    if prev is not None:
                        ptb, pp, pexps = prev
                        if g == 0:
                            av0 = ps.tile([128, 2, 512], f32, tag="work",
                                          bufs=2, name="av0")
                        if g < 8:
                            emit_av_chunk(pp, 0, pexps, av0, 2 * g, 2 * g + 2)
                        if g == 8:
                            avs0 = emit_av_merge(av0, 0)
                            av1 = ps.tile([128, 2, 512], f32, tag="work",
                                          bufs=2, name="av1")
                        if 8 <= g <= 13:
                            k0, k1 = av1_rng[g]
                            emit_av_chunk(pp, 1, pexps, av1, k0, k1)
                        if g == 14:
                            avs1 = emit_av_merge(av1, 1)
                    if s == 0:
                        emit_vproj(g)
                    if g == 3 and tb + 1 < TB and s > 0:
                        emit_qproj(tb + 1, p)
                    if g == 5 and s == 0:
                        emit_qproj(1, 0)
                    if g in (14, 15) and tb >= 1 and p >= 1 and prev is not None:
                        for ti_, eb_ in po_sched[p][
                            (0 if g == 14 else 2) : (2 if g == 14 else 3)
                        ]:
                            emit_outproj(tb - 1, ti_, eb_)
                if prev is not None:
                    ptb, pp, _ = prev
                    emit_norm(ptb, pp, avs0, avs1)
                prev = (tb, p, exps_cur)

            # epilogue
            ptb, pp, pexps = prev
            av0 = ps.tile([128, 2, 512], f32, tag="work", bufs=2, name="av0")
            emit_av_chunk(pp, 0, pexps, av0, 0, KC)
            avs0 = emit_av_merge(av0, 0)
            av1 = ps.tile([128, 2, 512], f32, tag="work", bufs=2, name="av1")
            emit_av_chunk(pp, 1, pexps, av1, 0, KC)
            avs1 = emit_av_merge(av1, 1)
            emit_norm(ptb, pp, avs0, avs1)
            for ti in range(4):
                for eb in range(2):
                    emit_outproj(TB - 1, ti, eb)

    return nc


_cache = {}


def _get_built():
    if "nc" not in _cache:
        nc = build_nc(S_FULL)
        nc.compile()
        _cache["nc"] = nc
    return _cache["nc"]


def shard_inputs(a, Wq, bq, Wk, Wv, Wo, S=S_FULL):
    import ml_dtypes

    bfnp = ml_dtypes.bfloat16
    in_maps = []
    for c in range(NCORES):
        b, hg = c // 2, c % 2
        sl = slice(hg * 512, (hg + 1) * 512)
        aT = np.ascontiguousarray(a[b].T).reshape(8, 128, S).astype(bfnp)
        wq_c = np.ascontiguousarray(Wq[:, sl]).reshape(8, 128, 512).astype(bfnp)
        wk_c = np.ascontiguousarray(Wk[:, sl]).reshape(8, 128, 512).astype(bfnp)
        wv_c = np.ascontiguousarray(Wv[:, sl]).reshape(8, 128, 512).astype(bfnp)
        wo_c = np.ascontiguousarray(Wo[sl, :]).reshape(4, 128, EMB).astype(bfnp)
        bq_c = np.ascontiguousarray(bq[sl].reshape(4, 128).T).astype(np.float32)
        in_maps.append(
            {"aT": aT, "wq": wq_c, "wk": wk_c, "wv": wv_c, "wo": wo_c, "bq": bq_c}
        )
    return in_maps


def kernel(a, Wq, bq, Wk, bk, Wv, bv, Wo, bo, trace=False):
    from concourse.bass_utils import run_bass_kernel_spmd

    a = np.asarray(a, np.float32)
    Wq = np.asarray(Wq, np.float32)
    bq = np.asarray(bq, np.float32)
    Wk = np.asarray(Wk, np.float32)
    Wv = np.asarray(Wv, np.float32)
    bv = np.asarray(bv, np.float32)
    Wo = np.asarray(Wo, np.float32)
    bo = np.asarray(bo, np.float32)

    nc = _get_built()
    in_maps = shard_inputs(a, Wq, bq, Wk, Wv, Wo)
    res = run_bass_kernel_spmd(nc, in_maps, list(range(NCORES)), trace=trace)
    _cache["last_result"] = res

    corr = (bo + bv @ Wo).astype(np.float32)
    out = np.empty((a.shape[0], S_FULL, EMB), np.float32)
    for b in range(a.shape[0]):
        out[b] = res.results[2 * b]["out"] + res.results[2 * b + 1]["out"] + corr[None, :]
    return out



# revision 2
# speedup vs baseline: 2.0175x; 2.0175x over previous
"""v13 (from v7): scores/AV keep K=64 A/B row-tiled pairs (concurrent PE
halves stream two rhs at once); Q/K/V/out PROJECTIONS switch to K=128
single-chain matmuls into plane 0 (PE pays ~2x stream on projections -- it has
~84us headroom under the ScalarE exp roof -- and DVE drops one merge op per
projection tile, ~53us).  reciprocal -> reciprocal_approx_fast (~5x).
Optional exp offload: n_off kc-groups per step have the h1 half of the score
pair computed as a bf16 cubic on DVE instead of ScalarE ACT (the critical
engine).  Softmax z rides the AV ones-column as in v7.

PSUM (8 banks): sc 2x[128,2,512]=4 + work 2x[128,2,512]=4.
"""

import numpy as np
from contextlib import ExitStack

EMB = 1024
NH_LOCAL = 8
NPAIR = 4
S_FULL = 2048
NCORES = 8
SCALE = float(np.sqrt(np.float32(EMB)))  # 32.0

# cubic fit of e^x, x = score/32 in [-1.6,1.6] (importance-weighted on the
# empirical score distribution); bf16-staged Horner validated end-to-end at
# 1.4e-3 max rel err with 4/16 tiles offloaded.
_C3, _C2, _C1, _C0 = 0.15933342, 0.52356269, 1.00347722, 0.99880712
PA = _C2 / _C3
PB = _C1 / _C3
PC = _C0 / _C3
PC3 = _C3

N_OFF = 0  # exp offload measured net-negative; ScalarE keeps all exps


def build_nc(S=S_FULL, reps=1, n_off=None):
    import concourse.bacc as bacc
    import concourse.tile as tile
    from concourse import mybir

    bf = mybir.dt.bfloat16
    f32 = mybir.dt.float32
    Act = mybir.ActivationFunctionType
    add = mybir.AluOpType.add
    mult = mybir.AluOpType.mult

    if n_off is None:
        n_off = N_OFF
    off_h1 = set(range(16 - n_off, 16))

    F = 512
    EC = EMB // 128
    T16 = S // 128
    TB = S // 512
    KC = S // 128

    nc = bacc.Bacc("TRN2", target_bir_lowering=False, debug=False)

    aT_d = nc.dram_tensor("aT", [EC, 128, S], bf, kind="ExternalInput")
    wq_d = nc.dram_tensor("wq", [EC, 128, F], bf, kind="ExternalInput")
    wk_d = nc.dram_tensor("wk", [EC, 128, F], bf, kind="ExternalInput")
    wv_d = nc.dram_tensor("wv", [EC, 128, F], bf, kind="ExternalInput")
    wo_d = nc.dram_tensor("wo", [NPAIR, 128, EMB], bf, kind="ExternalInput")
    bq_d = nc.dram_tensor("bq", [128, NPAIR], f32, kind="ExternalInput")
    out_d = nc.dram_tensor("out", [S, EMB], f32, kind="ExternalOutput")

    with ExitStack() as top:
        tc = top.enter_context(tile.TileContext(nc))
        sb = top.enter_context(tc.tile_pool(name="sb", bufs=1))
        ps = top.enter_context(tc.tile_pool(name="ps", bufs=1, space="PSUM"))

        wo_sb = sb.tile([128, NPAIR, EMB], bf)
        bq_sb = sb.tile([128, NPAIR], f32)
        ones_b = sb.tile([128, 65], bf)
        zr2 = sb.tile([128, 2, 512], bf)
        qt_sb = sb.tile([128, NPAIR, S], bf)
        kt_sb = sb.tile([128, NPAIR, S], bf)
        v_sb = sb.tile([128, T16, NH_LOCAL, 65], bf)
        ctxT = sb.tile([128, NPAIR, S], bf)
        aT = sb.tile([128, EC, S], bf)
        wq_sb = sb.tile([128, EC, F], bf)
        wk_sb = sb.tile([128, EC, F], bf)
        wv_sb = sb.tile([128, EC, F], bf)

        nc.vector.memset(ones_b[:], 1.0)
        nc.vector.memset(zr2[:], 0.0)
        nc.vector.memset(v_sb[:, :, :, 64:65], 1.0)
        for c in range(EC):
            nc.sync.dma_start(aT[:, c, :], aT_d[c])
            nc.sync.dma_start(wk_sb[:, c, :], wk_d[c])
        for c in range(EC):
            nc.sync.dma_start(wq_sb[:, c, :], wq_d[c])
            nc.sync.dma_start(wv_sb[:, c, :], wv_d[c])
        for p in range(NPAIR):
            nc.sync.dma_start(wo_sb[:, p, :], wo_d[p])
        nc.sync.dma_start(bq_sb[:], bq_d[:])

        A = slice(0, 64)
        B = slice(64, 128)

        def chain_k128(wk_tile, lhs_fn, rhs_fn, n):
            """K=128 full-array chain into plane 0 of a work tile."""
            for c in range(n):
                nc.tensor.matmul(
                    wk_tile[:, 0, :],
                    lhs_fn(c),
                    rhs_fn(c),
                    start=(c == 0),
                    stop=(c == n - 1),
                )

        def emit_vproj(t):
            wk = ps.tile([128, 2, 512], f32, tag="work", bufs=2, name="wk")
            chain_k128(
                wk,
                lambda c: aT[:, c, t * 128 : (t + 1) * 128],
                lambda c: wv_sb[:, c, :],
                EC,
            )
            nc.vector.tensor_copy(
                v_sb[:, t, :, 0:64],
                wk[:, 0, :].rearrange("p (h d) -> p h d", h=NH_LOCAL),
            )

        def emit_qproj(tb, p):
            ts = slice(tb * 512, (tb + 1) * 512)
            wk = ps.tile([128, 2, 512], f32, tag="work", bufs=2, name="wk")
            chain_k128(
                wk,
                lambda c: wq_sb[:, c, p * 128 : (p + 1) * 128],
                lambda c: aT[:, c, ts],
                EC,
            )
            nc.vector.tensor_scalar_add(
                qt_sb[:, p, ts], wk[:, 0, :], bq_sb[:, p : p + 1]
            )

        def emit_kproj(tb, p):
            ts = slice(tb * 512, (tb + 1) * 512)
            wk = ps.tile([128, 2, 512], f32, tag="work", bufs=2, name="wk")
            chain_k128(
                wk,
                lambda c: wk_sb[:, c, p * 128 : (p + 1) * 128],
                lambda c: aT[:, c, ts],
                EC,
            )
            nc.vector.tensor_copy(kt_sb[:, p, ts], wk[:, 0, :])

        def emit_score_group(tb, p, kc, exps):
            ts = slice(tb * 512, (tb + 1) * 512)
            ks = slice(kc * 128, (kc + 1) * 128)
            sc = ps.tile([128, 2, 512], f32, tag="sc", bufs=2)
            nc.tensor.matmul(
                sc[:, 0, :], kt_sb[A, p, ks], qt_sb[A, p, ts], start=True, stop=True
            )
            nc.tensor.matmul(
                sc[:, 1, :], kt_sb[B, p, ks], qt_sb[B, p, ts], start=True, stop=True
            )
            if kc in off_h1:
                nc.scalar.activation(
                    exps[:, kc, 0, :], sc[:, 0, :], Act.Exp, scale=1.0 / SCALE
                )
                x16 = sb.tile([128, 512], bf, tag="poly", bufs=3, name="x16")
                g1 = sb.tile([128, 512], bf, tag="poly", bufs=3, name="g1")
                g2 = sb.tile([128, 512], bf, tag="poly", bufs=3, name="g2")
                nc.vector.tensor_scalar_mul(x16[:], sc[:, 1, :], 1.0 / SCALE)
                nc.vector.scalar_tensor_tensor(
                    g1[:], x16[:], PA, x16[:], add, mult
                )
                nc.vector.scalar_tensor_tensor(
                    g2[:], g1[:], PB, x16[:], add, mult
                )
                nc.vector.tensor_scalar(
                    exps[:, kc, 1, :], g2[:], PC, PC3, add, mult
                )
            else:
                nc.scalar.activation(
                    exps[:, kc, :, :], sc[:], Act.Exp, scale=1.0 / SCALE
                )

        def emit_av_chunk(p, hh, exps, av, k0, k1):
            h = 2 * p + hh
            for kc in range(k0, k1):
                for j, hs in ((0, A), (1, B)):
                    nc.tensor.matmul(
                        av[0:65, j, :],
                        v_sb[hs, kc, h, :],
                        exps[hs, kc, hh, :],
                        start=(kc == 0),
                        stop=(kc == KC - 1),
                    )

        def emit_av_merge(av, hh):
            """avs = av.planeA + av.planeB via two 1-PSUM-input DVE ops."""
            avs = sb.tile([65, 512], f32, tag="avs", bufs=2)
            nc.vector.tensor_copy(avs[:], av[0:65, 0, :])
            nc.vector.tensor_add(avs[:], avs[:], av[0:65, 1, :])
            nc.vector.tensor_copy(zr2[64:65, hh, :], avs[64:65, :])
            return avs

        def emit_norm(tb, p, avs0, avs1):
            ts = slice(tb * 512, (tb + 1) * 512)
            zw = ps.tile([128, 2, 512], f32, tag="work", bufs=2, name="zw")
            for hh in range(2):
                nc.tensor.matmul(
                    zw[0:65, hh, :],
                    ones_b[B, 0:65],
                    zr2[B, hh, :],
                    start=True,
                    stop=True,
                )
            rb = sb.tile([64, 2, 512], f32, tag="rb", bufs=1)
            for hh in range(2):
                nc.vector.reciprocal_approx_fast(rb[:, hh, :], zw[0:64, hh, :])
            for hh, avs in ((0, avs0), (1, avs1)):
                nc.vector.tensor_mul(
                    ctxT[hh * 64 : (hh + 1) * 64, p, ts],
                    avs[0:64, :],
                    rb[:, hh, :],
                )

        def emit_outproj(tb, ti_local, eb):
            t = tb * 4 + ti_local
            wk = ps.tile([128, 2, 512], f32, tag="work", bufs=2, name="wk")
            chain_k128(
                wk,
                lambda p_: ctxT[:, p_, t * 128 : (t + 1) * 128],
                lambda p_: wo_sb[:, p_, eb * 512 : (eb + 1) * 512],
                NPAIR,
            )
            o_sb = sb.tile([128, 512], f32, tag="ost", bufs=1)
            nc.vector.tensor_copy(o_sb[:], wk[:, 0, :])
            nc.sync.dma_start(
                out_d[t * 128 : (t + 1) * 128, eb * 512 : (eb + 1) * 512],
                o_sb[:],
            )

        for _rep in range(reps):
            for p in range(NPAIR):
                emit_kproj(0, p)
            for p in range(NPAIR):
                emit_qproj(0, p)

            steps = [(tb, p) for tb in range(TB) for p in range(NPAIR)]
            po_sched = {1: [(0, 0), (0, 1), (1, 0)], 2: [(1, 1), (2, 0), (2, 1)],
                        3: [(3, 0), (3, 1)]}
            av1_rng = {8: (0, 3), 9: (3, 6), 10: (6, 9), 11: (9, 12),
                       12: (12, 14), 13: (14, 16)}
            prev = None
            for s, (tb, p) in enumerate(steps):
                exps_cur = sb.tile([128, KC, 2, 512], bf, tag="exps", bufs=2)
                av0 = av1 = avs0 = avs1 = None
                for g in range(16):
                    if s == 0 and g in (4, 8, 12):
                        for pp_ in range(NPAIR):
                            emit_kproj(g // 4, pp_)
                    emit_score_group(tb, p, g, exps_cur)
                    if prev is not None:
                        ptb, pp, pexps = prev
                        if g == 0:
                            av0 = ps.tile([128, 2, 512], f32, tag="work",
                                          bufs=2, name="av0")
                        if g < 8:
                            emit_av_chunk(pp, 0, pexps, av0, 2 * g, 2 * g + 2)
                        if g == 8:
                            avs0 = emit_av_merge(av0, 0)
                            av1 = ps.tile([128, 2, 512], f32, tag="work",
                                          bufs=2, name="av1")
                        if 8 <= g <= 13:
                            k0, k1 = av1_rng[g]
                            emit_av_chunk(pp, 1, pexps, av1, k0, k1)
                        if g == 14:
                            avs1 = emit_av_merge(av1, 1)
                    if s == 0:
                        emit_vproj(g)
                    if g == 3 and tb + 1 < TB and s > 0:
                        emit_qproj(tb + 1, p)
                    if g == 5 and s == 0:
                        emit_qproj(1, 0)
                    if g in (14, 15) and tb >= 1 and p >= 1 and prev is not None:
                        for ti_, eb_ in po_sched[p][
                            (0 if g == 14 else 2) : (2 if g == 14 else 3)
                        ]:
                            emit_outproj(tb - 1, ti_, eb_)
                if prev is not None:
                    ptb, pp, _ = prev
                    emit_norm(ptb, pp, avs0, avs1)
                prev = (tb, p, exps_cur)

            # epilogue
            ptb, pp, pexps = prev
            av0 = ps.tile([128, 2, 512], f32, tag="work", bufs=2, name="av0")
            emit_av_chunk(pp, 0, pexps, av0, 0, KC)
            avs0 = emit_av_merge(av0, 0)
            av1 = ps.tile([128, 2, 512], f32, tag="work", bufs=2, name="av1")
            emit_av_chunk(pp, 1, pexps, av1, 0, KC)
            avs1 = emit_av_merge(av1, 1)
            emit_norm(ptb, pp, avs0, avs1)
            for ti in range(4):
                for eb in range(2):
                    emit_outproj(TB - 1, ti, eb)

    return nc


_cache = {}


def _get_built():
    if "nc" not in _cache:
        nc = build_nc(S_FULL)
        nc.compile()
        _cache["nc"] = nc
    return _cache["nc"]


def shard_inputs(a, Wq, bq, Wk, Wv, Wo, S=S_FULL):
    import ml_dtypes

    bfnp = ml_dtypes.bfloat16
    in_maps = []
    for c in range(NCORES):
        b, hg = c // 2, c % 2
        sl = slice(hg * 512, (hg + 1) * 512)
        aT = np.ascontiguousarray(a[b].T).reshape(8, 128, S).astype(bfnp)
        wq_c = np.ascontiguousarray(Wq[:, sl]).reshape(8, 128, 512).astype(bfnp)
        wk_c = np.ascontiguousarray(Wk[:, sl]).reshape(8, 128, 512).astype(bfnp)
        wv_c = np.ascontiguousarray(Wv[:, sl]).reshape(8, 128, 512).astype(bfnp)
        wo_c = np.ascontiguousarray(Wo[sl, :]).reshape(4, 128, EMB).astype(bfnp)
        bq_c = np.ascontiguousarray(bq[sl].reshape(4, 128).T).astype(np.float32)
        in_maps.append(
            {"aT": aT, "wq": wq_c, "wk": wk_c, "wv": wv_c, "wo": wo_c, "bq": bq_c}
        )
    return in_maps


def kernel(a, Wq, bq, Wk, bk, Wv, bv, Wo, bo, trace=False):
    from concourse.bass_utils import run_bass_kernel_spmd

    a = np.asarray(a, np.float32)
    Wq = np.asarray(Wq, np.float32)
    bq = np.asarray(bq, np.float32)
    Wk = np.asarray(Wk, np.float32)
    Wv = np.asarray(Wv, np.float32)
    bv = np.asarray(bv, np.float32)
    Wo = np.asarray(Wo, np.float32)
    bo = np.asarray(bo, np.float32)

    nc = _get_built()
    in_maps = shard_inputs(a, Wq, bq, Wk, Wv, Wo)
    res = run_bass_kernel_spmd(nc, in_maps, list(range(NCORES)), trace=trace)
    _cache["last_result"] = res

    corr = (bo + bv @ Wo).astype(np.float32)
    out = np.empty((a.shape[0], S_FULL, EMB), np.float32)
    for b in range(a.shape[0]):
        out[b] = res.results[2 * b]["out"] + res.results[2 * b + 1]["out"] + corr[None, :]
    return out



# revision 3
# speedup vs baseline: 2.5700x; 1.2739x over previous
"""v14 (from v13: kproj prologue spread 1-per-group, ost bufs=2)

v13 (from v7): scores/AV keep K=64 A/B row-tiled pairs (concurrent PE
halves stream two rhs at once); Q/K/V/out PROJECTIONS switch to K=128
single-chain matmuls into plane 0 (PE pays ~2x stream on projections -- it has
~84us headroom under the ScalarE exp roof -- and DVE drops one merge op per
projection tile, ~53us).  reciprocal -> reciprocal_approx_fast (~5x).
Optional exp offload: n_off kc-groups per step have the h1 half of the score
pair computed as a bf16 cubic on DVE instead of ScalarE ACT (the critical
engine).  Softmax z rides the AV ones-column as in v7.

PSUM (8 banks): sc 2x[128,2,512]=4 + work 2x[128,2,512]=4.
"""

import numpy as np
from contextlib import ExitStack

EMB = 1024
NH_LOCAL = 8
NPAIR = 4
S_FULL = 2048
NCORES = 8
SCALE = float(np.sqrt(np.float32(EMB)))  # 32.0

# cubic fit of e^x, x = score/32 in [-1.6,1.6] (importance-weighted on the
# empirical score distribution); bf16-staged Horner validated end-to-end at
# 1.4e-3 max rel err with 4/16 tiles offloaded.
_C3, _C2, _C1, _C0 = 0.15933342, 0.52356269, 1.00347722, 0.99880712
PA = _C2 / _C3
PB = _C1 / _C3
PC = _C0 / _C3
PC3 = _C3

N_OFF = 0  # exp offload measured net-negative; ScalarE keeps all exps


def build_nc(S=S_FULL, reps=1, n_off=None):
    import concourse.bacc as bacc
    import concourse.tile as tile
    from concourse import mybir

    bf = mybir.dt.bfloat16
    f32 = mybir.dt.float32
    Act = mybir.ActivationFunctionType
    add = mybir.AluOpType.add
    mult = mybir.AluOpType.mult

    if n_off is None:
        n_off = N_OFF
    off_h1 = set(range(16 - n_off, 16))

    F = 512
    EC = EMB // 128
    T16 = S // 128
    TB = S // 512
    KC = S // 128

    nc = bacc.Bacc("TRN2", target_bir_lowering=False, debug=False)

    aT_d = nc.dram_tensor("aT", [EC, 128, S], bf, kind="ExternalInput")
    wq_d = nc.dram_tensor("wq", [EC, 128, F], bf, kind="ExternalInput")
    wk_d = nc.dram_tensor("wk", [EC, 128, F], bf, kind="ExternalInput")
    wv_d = nc.dram_tensor("wv", [EC, 128, F], bf, kind="ExternalInput")
    wo_d = nc.dram_tensor("wo", [NPAIR, 128, EMB], bf, kind="ExternalInput")
    bq_d = nc.dram_tensor("bq", [128, NPAIR], f32, kind="ExternalInput")
    out_d = nc.dram_tensor("out", [S, EMB], f32, kind="ExternalOutput")

    with ExitStack() as top:
        tc = top.enter_context(tile.TileContext(nc))
        sb = top.enter_context(tc.tile_pool(name="sb", bufs=1))
        ps = top.enter_context(tc.tile_pool(name="ps", bufs=1, space="PSUM"))

        wo_sb = sb.tile([128, NPAIR, EMB], bf)
        bq_sb = sb.tile([128, NPAIR], f32)
        ones_b = sb.tile([128, 65], bf)
        zr2 = sb.tile([128, 2, 512], bf)
        qt_sb = sb.tile([128, NPAIR, S], bf)
        kt_sb = sb.tile([128, NPAIR, S], bf)
        v_sb = sb.tile([128, T16, NH_LOCAL, 65], bf)
        ctxT = sb.tile([128, NPAIR, S], bf)
        aT = sb.tile([128, EC, S], bf)
        wq_sb = sb.tile([128, EC, F], bf)
        wk_sb = sb.tile([128, EC, F], bf)
        wv_sb = sb.tile([128, EC, F], bf)

        nc.vector.memset(ones_b[:], 1.0)
        nc.vector.memset(zr2[:], 0.0)
        nc.vector.memset(v_sb[:, :, :, 64:65], 1.0)
        for c in range(EC):
            nc.sync.dma_start(aT[:, c, :], aT_d[c])
            nc.sync.dma_start(wk_sb[:, c, :], wk_d[c])
        for c in range(EC):
            nc.sync.dma_start(wq_sb[:, c, :], wq_d[c])
            nc.sync.dma_start(wv_sb[:, c, :], wv_d[c])
        for p in range(NPAIR):
            nc.sync.dma_start(wo_sb[:, p, :], wo_d[p])
        nc.sync.dma_start(bq_sb[:], bq_d[:])

        A = slice(0, 64)
        B = slice(64, 128)

        def chain_k128(wk_tile, lhs_fn, rhs_fn, n):
            """K=128 full-array chain into plane 0 of a work tile."""
            for c in range(n):
                nc.tensor.matmul(
                    wk_tile[:, 0, :],
                    lhs_fn(c),
                    rhs_fn(c),
                    start=(c == 0),
                    stop=(c == n - 1),
                )

        def emit_vproj(t):
            wk = ps.tile([128, 2, 512], f32, tag="work", bufs=2, name="wk")
            chain_k128(
                wk,
                lambda c: aT[:, c, t * 128 : (t + 1) * 128],
                lambda c: wv_sb[:, c, :],
                EC,
            )
            nc.vector.tensor_copy(
                v_sb[:, t, :, 0:64],
                wk[:, 0, :].rearrange("p (h d) -> p h d", h=NH_LOCAL),
            )

        def emit_qproj(tb, p):
            ts = slice(tb * 512, (tb + 1) * 512)
            wk = ps.tile([128, 2, 512], f32, tag="work", bufs=2, name="wk")
            chain_k128(
                wk,
                lambda c: wq_sb[:, c, p * 128 : (p + 1) * 128],
                lambda c: aT[:, c, ts],
                EC,
            )
            nc.vector.tensor_scalar_add(
                qt_sb[:, p, ts], wk[:, 0, :], bq_sb[:, p : p + 1]
            )

        def emit_kproj(tb, p):
            ts = slice(tb * 512, (tb + 1) * 512)
            wk = ps.tile([128, 2, 512], f32, tag="work", bufs=2, name="wk")
            chain_k128(
                wk,
                lambda c: wk_sb[:, c, p * 128 : (p + 1) * 128],
                lambda c: aT[:, c, ts],
                EC,
            )
            nc.vector.tensor_copy(kt_sb[:, p, ts], wk[:, 0, :])

        def emit_score_group(tb, p, kc, exps):
            ts = slice(tb * 512, (tb + 1) * 512)
            ks = slice(kc * 128, (kc + 1) * 128)
            sc = ps.tile([128, 2, 512], f32, tag="sc", bufs=2)
            nc.tensor.matmul(
                sc[:, 0, :], kt_sb[A, p, ks], qt_sb[A, p, ts], start=True, stop=True
            )
            nc.tensor.matmul(
                sc[:, 1, :], kt_sb[B, p, ks], qt_sb[B, p, ts], start=True, stop=True
            )
            if kc in off_h1:
                nc.scalar.activation(
                    exps[:, kc, 0, :], sc[:, 0, :], Act.Exp, scale=1.0 / SCALE
                )
                x16 = sb.tile([128, 512], bf, tag="poly", bufs=3, name="x16")
                g1 = sb.tile([128, 512], bf, tag="poly", bufs=3, name="g1")
                g2 = sb.tile([128, 512], bf, tag="poly", bufs=3, name="g2")
                nc.vector.tensor_scalar_mul(x16[:], sc[:, 1, :], 1.0 / SCALE)
                nc.vector.scalar_tensor_tensor(
                    g1[:], x16[:], PA, x16[:], add, mult
                )
                nc.vector.scalar_tensor_tensor(
                    g2[:], g1[:], PB, x16[:], add, mult
                )
                nc.vector.tensor_scalar(
                    exps[:, kc, 1, :], g2[:], PC, PC3, add, mult
                )
            else:
                nc.scalar.activation(
                    exps[:, kc, :, :], sc[:], Act.Exp, scale=1.0 / SCALE
                )

        def emit_av_chunk(p, hh, exps, av, k0, k1):
            h = 2 * p + hh
            for kc in range(k0, k1):
                for j, hs in ((0, A), (1, B)):
                    nc.tensor.matmul(
                        av[0:65, j, :],
                        v_sb[hs, kc, h, :],
                        exps[hs, kc, hh, :],
                        start=(kc == 0),
                        stop=(kc == KC - 1),
                    )

        def emit_av_merge(av, hh):
            """avs = av.planeA + av.planeB via two 1-PSUM-input DVE ops."""
            avs = sb.tile([65, 512], f32, tag="avs", bufs=2)
            nc.vector.tensor_copy(avs[:], av[0:65, 0, :])
            nc.vector.tensor_add(avs[:], avs[:], av[0:65, 1, :])
            nc.vector.tensor_copy(zr2[64:65, hh, :], avs[64:65, :])
            return avs

        def emit_norm(tb, p, avs0, avs1):
            ts = slice(tb * 512, (tb + 1) * 512)
            zw = ps.tile([128, 2, 512], f32, tag="work", bufs=2, name="zw")
            for hh in range(2):
                nc.tensor.matmul(
                    zw[0:65, hh, :],
                    ones_b[B, 0:65],
                    zr2[B, hh, :],
                    start=True,
                    stop=True,
                )
            rb = sb.tile([64, 2, 512], f32, tag="rb", bufs=1)
            for hh in range(2):
                nc.vector.reciprocal_approx_fast(rb[:, hh, :], zw[0:64, hh, :])
            for hh, avs in ((0, avs0), (1, avs1)):
                nc.vector.tensor_mul(
                    ctxT[hh * 64 : (hh + 1) * 64, p, ts],
                    avs[0:64, :],
                    rb[:, hh, :],
                )

        def emit_outproj(tb, ti_local, eb):
            t = tb * 4 + ti_local
            wk = ps.tile([128, 2, 512], f32, tag="work", bufs=2, name="wk")
            chain_k128(
                wk,
                lambda p_: ctxT[:, p_, t * 128 : (t + 1) * 128],
                lambda p_: wo_sb[:, p_, eb * 512 : (eb + 1) * 512],
                NPAIR,
            )
            o_sb = sb.tile([128, 512], f32, tag="ost", bufs=2)
            nc.vector.tensor_copy(o_sb[:], wk[:, 0, :])
            nc.sync.dma_start(
                out_d[t * 128 : (t + 1) * 128, eb * 512 : (eb + 1) * 512],
                o_sb[:],
            )

        for _rep in range(reps):
            for p in range(NPAIR):
                emit_kproj(0, p)
            for p in range(NPAIR):
                emit_qproj(0, p)

            steps = [(tb, p) for tb in range(TB) for p in range(NPAIR)]
            po_sched = {1: [(0, 0), (0, 1), (1, 0)], 2: [(1, 1), (2, 0), (2, 1)],
                        3: [(3, 0), (3, 1)]}
            av1_rng = {8: (0, 3), 9: (3, 6), 10: (6, 9), 11: (9, 12),
                       12: (12, 14), 13: (14, 16)}
            prev = None
            for s, (tb, p) in enumerate(steps):
                exps_cur = sb.tile([128, KC, 2, 512], bf, tag="exps", bufs=2)
                av0 = av1 = avs0 = avs1 = None
                for g in range(16):
                    if s == 0 and g >= 4:
                        emit_kproj(g // 4, g % 4)
                    emit_score_group(tb, p, g, exps_cur)
                    if prev is not None:
                        ptb, pp, pexps = prev
                        if g == 0:
                            av0 = ps.tile([128, 2, 512], f32, tag="work",
                                          bufs=2, name="av0")
                        if g < 8:
                            emit_av_chunk(pp, 0, pexps, av0, 2 * g, 2 * g + 2)
                        if g == 8:
                            avs0 = emit_av_merge(av0, 0)
                            av1 = ps.tile([128, 2, 512], f32, tag="work",
                                          bufs=2, name="av1")
                        if 8 <= g <= 13:
                            k0, k1 = av1_rng[g]
                            emit_av_chunk(pp, 1, pexps, av1, k0, k1)
                        if g == 14:
                            avs1 = emit_av_merge(av1, 1)
                    if s == 0:
                        emit_vproj(g)
                    if g == 3 and tb + 1 < TB and s > 0:
                        emit_qproj(tb + 1, p)
                    if g == 5 and s == 0:
                        emit_qproj(1, 0)
                    if g in (14, 15) and tb >= 1 and p >= 1 and prev is not None:
                        for ti_, eb_ in po_sched[p][
                            (0 if g == 14 else 2) : (2 if g == 14 else 3)
                        ]:
                            emit_outproj(tb - 1, ti_, eb_)
                if prev is not None:
                    ptb, pp, _ = prev
                    emit_norm(ptb, pp, avs0, avs1)
                prev = (tb, p, exps_cur)

            # epilogue
            ptb, pp, pexps = prev
            av0 = ps.tile([128, 2, 512], f32, tag="work", bufs=2, name="av0")
            emit_av_chunk(pp, 0, pexps, av0, 0, KC)
            avs0 = emit_av_merge(av0, 0)
            av1 = ps.tile([128, 2, 512], f32, tag="work", bufs=2, name="av1")
            emit_av_chunk(pp, 1, pexps, av1, 0, KC)
            avs1 = emit_av_merge(av1, 1)
            emit_norm(ptb, pp, avs0, avs1)
            for ti in range(4):
                for eb in range(2):
                    emit_outproj(TB - 1, ti, eb)

    return nc


_cache = {}


def _get_built():
    if "nc" not in _cache:
        nc = build_nc(S_FULL)
        nc.compile()
        _cache["nc"] = nc
    return _cache["nc"]


def shard_inputs(a, Wq, bq, Wk, Wv, Wo, S=S_FULL):
    import ml_dtypes

    bfnp = ml_dtypes.bfloat16
    in_maps = []
    for c in range(NCORES):
        b, hg = c // 2, c % 2
        sl = slice(hg * 512, (hg + 1) * 512)
        aT = np.ascontiguousarray(a[b].T).reshape(8, 128, S).astype(bfnp)
        wq_c = np.ascontiguousarray(Wq[:, sl]).reshape(8, 128, 512).astype(bfnp)
        wk_c = np.ascontiguousarray(Wk[:, sl]).reshape(8, 128, 512).astype(bfnp)
        wv_c = np.ascontiguousarray(Wv[:, sl]).reshape(8, 128, 512).astype(bfnp)
        wo_c = np.ascontiguousarray(Wo[sl, :]).reshape(4, 128, EMB).astype(bfnp)
        bq_c = np.ascontiguousarray(bq[sl].reshape(4, 128).T).astype(np.float32)
        in_maps.append(
            {"aT": aT, "wq": wq_c, "wk": wk_c, "wv": wv_c, "wo": wo_c, "bq": bq_c}
        )
    return in_maps


def kernel(a, Wq, bq, Wk, bk, Wv, bv, Wo, bo, trace=False):
    from concourse.bass_utils import run_bass_kernel_spmd

    a = np.asarray(a, np.float32)
    Wq = np.asarray(Wq, np.float32)
    bq = np.asarray(bq, np.float32)
    Wk = np.asarray(Wk, np.float32)
    Wv = np.asarray(Wv, np.float32)
    bv = np.asarray(bv, np.float32)
    Wo = np.asarray(Wo, np.float32)
    bo = np.asarray(bo, np.float32)

    nc = _get_built()
    in_maps = shard_inputs(a, Wq, bq, Wk, Wv, Wo)
    res = run_bass_kernel_spmd(nc, in_maps, list(range(NCORES)), trace=trace)
    _cache["last_result"] = res

    corr = (bo + bv @ Wo).astype(np.float32)
    out = np.empty((a.shape[0], S_FULL, EMB), np.float32)
    for b in range(a.shape[0]):
        out[b] = res.results[2 * b]["out"] + res.results[2 * b + 1]["out"] + corr[None, :]
    return out

